# revision 61
# baseline (speedup 1.0000x reference)
"""Dense transformer block on 8 Trainium2 NeuronCores — fp8 DoubleRow edition.

Sharding: each core owns half a batch element (512 rows out of [4, 1024, C]).
Cores redundantly compute LN1 + K/V projections for the full batch element
(so attention needs no cross-core communication); Q / attention / proj / MLP
run only on the core's own 512 rows.  No collectives.

Precision plan (rel-l2 budget 2e-2; this lands ~6e-3):
  - x streamed as bf16 for the LN1 path; fp32 rows reloaded for the residual.
  - attention-side matmuls (q/k/v proj, attn@V, proj) in fp8e4m3 DoubleRow
    (2 k-tiles per pass, 0.5 PE cycles/row): activations scaled x32, weights
    x1024 (q-weights x8192 with softmax scale folded), compensated in the
    PSUM->SBUF copies.  q/k kept fp32r; scores matmul fp32r; exp output
    quantized to fp8 (exp(s)/4 to dodge e4m3 saturation at 240) — softmax
    renormalization by the appended ones-column cancels P-quantization scale.
  - MLP (fc1/gelu/fc2) in bf16 end-to-end: same PE rate as fp32r, half DMA.

Host-side prep:
  - weights pre-transposed to [in_feat, out_feat], LN gammas folded into the
    following matmul weights, betas into biases; q scale folded into W_q/b_q;
    k bias dropped (softmax shift-invariant), v bias folded into proj bias;
    proj bias folded into the residual rows (xrb = x_own + bp).
  - fp8 weights packed in DoubleRow plane-pair layout [128, k-pair, 2, out].
  - per-core x rows permuted to [own 512 | other 512]; softmax is invariant
    to key/value ordering so attention over permuted K/V is exact.
"""

import sys

if "/opt/trn_rl_repo" not in sys.path:
    sys.path.insert(0, "/opt/trn_rl_repo")

import numpy as np
import ml_dtypes

import concourse.bacc as bacc
import concourse.bass as bass
import concourse.mybir as mybir
import concourse.tile as tile
from concourse.masks import make_identity

FP = mybir.dt.float32
FPR = mybir.dt.float32r
BF = mybir.dt.bfloat16
F8 = mybir.dt.float8e4
AF = mybir.ActivationFunctionType
DR = mybir.MatmulPerfMode.DoubleRow

NP_BF = ml_dtypes.bfloat16
NP_F8 = ml_dtypes.float8_e4m3

N_CORES = 8
P = 128
C = 1024            # model dim
H = 16              # heads
HD = 64             # head dim
HID = 4096          # mlp hidden
N_ALL = 1024        # rows per batch element
N_OWN = 512         # rows owned per core
EPS = 1e-5

CT = C // P         # 8 feature chunks of 128
CJ = CT // 2        # 4 DoubleRow plane-pairs
NT_ALL = N_ALL // P # 8 row tiles
NT_OWN = N_OWN // P # 4 row tiles

# quantization scales (powers of two; compensated in PSUM->SBUF copies)
SH = 32.0           # fp8 activation scale (h, v, o)
SW = 1024.0         # fp8 weight scale (wk, wv, wp)
SWQ = 8192.0        # fp8 q-weight scale (softmax 1/8 folded in first)
LN2_2 = float(2.0 * np.log(2.0))


def _bcast(ap, p=P):
    """Partition-broadcast a [*free] AP to [p, *free] (step-0 partition dim)."""
    return bass.AP(tensor=ap.tensor, offset=ap.offset, ap=[[0, p], *ap.ap])


def _layernorm_tile(nc, pool, out, xt, eps_sb, sqrt_scale, ts_eng=None):
    """out = (xt - mean) * (1/sqrt_scale) / sqrt(var + eps), rowwise over C.

    eps_sb must hold EPS * sqrt_scale^2 so that
    rstd = 1 / sqrt(sqrt_scale^2 * var + eps * sqrt_scale^2)
         = (1/sqrt_scale) / sqrt(var + eps).
    Used with sqrt_scale=2^-5 to get an extra x32 folded into the output.
    """
    st = pool.tile([P, 2, 6], FP, tag="ln_st", name="ln_st")
    for g in range(2):
        nc.vector.bn_stats(out=st[:, g, :], in_=xt[:, 512 * g:512 * (g + 1)])
    mv = pool.tile([P, 2], FP, tag="ln_mv", name="ln_mv")
    nc.vector.bn_aggr(out=mv, in_=st)
    rstd = pool.tile([P, 1], FP, tag="ln_rstd", name="ln_rstd")
    nc.scalar.activation(out=rstd, in_=mv[:, 1:2], func=AF.Sqrt, bias=eps_sb,
                         scale=sqrt_scale * sqrt_scale)
    nc.vector.reciprocal(out=rstd, in_=rstd)
    ts = ts_eng if ts_eng is not None else nc.vector
    for g in range(2):
        sl = slice(512 * g, 512 * (g + 1))
        ts.tensor_scalar(
            out=out[:, sl], in0=xt[:, sl],
            scalar1=mv[:, 0:1], scalar2=rstd,
            op0=mybir.AluOpType.subtract, op1=mybir.AluOpType.mult,
        )


def build():
    nc = bacc.Bacc("TRN2", target_bir_lowering=False, debug=False,
                   num_devices=N_CORES)

    x_d = nc.dram_tensor("x", [N_ALL, C], BF, kind="ExternalInput")
    xrb_d = nc.dram_tensor("xrb", [N_OWN, C], BF, kind="ExternalInput")
    wqk_d = nc.dram_tensor("wqk", [CT, P, CJ, 2, 256], F8, kind="ExternalInput")
    wv_d = nc.dram_tensor("wv", [2, P, CJ, 2, 512], F8, kind="ExternalInput")
    bq_d = nc.dram_tensor("bq", [P, CT], FP, kind="ExternalInput")
    wp_d = nc.dram_tensor("wp", [P, CJ, 2, C], F8, kind="ExternalInput")
    w1_d = nc.dram_tensor("w1", [8, P, 2, CJ, 2, 512], F8, kind="ExternalInput")
    b2_d = nc.dram_tensor("b2", [P, HID // P], FP, kind="ExternalInput")
    wf2_d = nc.dram_tensor("wf2", [4, P, 4, 2, 2, C], F8, kind="ExternalInput")
    bf2_d = nc.dram_tensor("bf2", [C], FP, kind="ExternalInput")
    out_d = nc.dram_tensor("out", [N_OWN, C], FP, kind="ExternalOutput")

    with tile.TileContext(nc, pool_alloc_mode="queue") as tc:
        consts = tc.alloc_tile_pool(name="consts", bufs=1)
        identb = consts.tile([P, P], BF)
        make_identity(nc, identb)
        bq_sb = consts.tile([P, CT], FP)
        nc.sync.dma_start(out=bq_sb, in_=bq_d[:])
        b2_sb = consts.tile([P, HID // P], FP)
        nc.sync.dma_start(out=b2_sb, in_=b2_d[:])
        bf2_fp = consts.tile([1, C], FP)
        nc.sync.dma_start(out=bf2_fp, in_=bf2_d[:].rearrange("(o c) -> o c", o=1))
        bf2_row = consts.tile([1, C], FPR)      # PE-broadcast in Phase G
        nc.vector.tensor_copy(out=bf2_row, in_=bf2_fp)
        eps1_sb = consts.tile([P, 1], FP)       # for LN1: eps * (2^-5)^2
        nc.vector.memset(eps1_sb, EPS / (SH * SH))
        eps2_sb = consts.tile([P, 1], FP)       # for LN2: plain eps
        nc.vector.memset(eps2_sb, EPS)
        nexp_b = consts.tile([P, 1], FP)        # exp bias: -2ln2 -> exp(s)/4
        nc.vector.memset(nexp_b, -LN2_2)
        ones_row = consts.tile([1, P], FPR)
        nc.vector.memset(ones_row.bitcast(FP), 1.0)

        # fc2 weights (hi+lo fp8, 4 hid-pairs per tile): fully resident;
        # DMA triggers paced through the fc1 loop.
        wf2_pool = tc.alloc_tile_pool(name="wf2", bufs=4, side="right")
        wf2_t = [wf2_pool.tile([P, 4, 2, 2, C], F8, tag="wf2", name="wf2")
                 for _ in range(4)]

        # Long-lived pools allocated up-front in reverse order of death so
        # every release happens at the top of the LIFO pool stack:
        # x2 dies at phase G end, h28/r28/wF at F end, wD/xres at D end.
        x2_pool = tc.alloc_tile_pool(name="x2", bufs=NT_OWN)
        h28_pool = tc.alloc_tile_pool(name="h28", bufs=CJ)
        r28_pool = tc.alloc_tile_pool(name="r28", bufs=CJ)
        wF = tc.alloc_tile_pool(name="wF", bufs=3)
        wD = tc.alloc_tile_pool(name="wD", bufs=1)
        xres = tc.alloc_tile_pool(name="xres", bufs=NT_OWN)
        # fc1 weights (hi+lo fp8): ring of 3 group-tiles; first 3 DMAs paced
        # through the attention loop, the rest triggered as fc1 consumes.
        w1_t = [wF.tile([P, 2, CJ, 2, 512], F8, tag="wF", name="wF")
                for _ in range(8)]

        # ---- Phase A: load x (bf16), LN1 -> h fp8 (x32), transpose ----
        h8_pool = tc.alloc_tile_pool(name="h8", bufs=CJ)
        pa = tc.alloc_tile_pool(name="pa", bufs=4)
        ha = tc.alloc_tile_pool(name="ha", bufs=2)
        psA = tc.alloc_tile_pool(name="psA", bufs=6, space="PSUM")
        h8 = [h8_pool.tile([P, 2, N_ALL], F8, tag="h8", name="h8")
              for _ in range(CJ)]
        x_tiles = []
        for i in range(NT_ALL):
            xt = pa.tile([P, C], BF, tag="x_in", name="x_in")
            nc.gpsimd.dma_start(out=xt, in_=x_d[P * i:P * (i + 1), :])
            x_tiles.append(xt)
        for i in range(NT_ALL):
            ht = ha.tile([P, C], BF, tag="h", name="h")
            _layernorm_tile(nc, pa, ht, x_tiles[i], eps1_sb, 1.0 / SH)
            for j in range(CJ):       # transpose chunk pairs, copy as one
                ps = psA.tile([P, 2, P], BF, tag="psA", name="psA")
                for t in range(2):
                    nc.tensor.transpose(ps[:, t, :],
                                        ht[:, P * (2 * j + t):P * (2 * j + t + 1)],
                                        identb)
                dst = h8[j][:, :, P * i:P * (i + 1)]
                if j % 2 == 0:
                    nc.vector.tensor_copy(out=dst, in_=ps)
                else:
                    nc.scalar.activation(out=dst, in_=ps, func=AF.Copy)
        ha.release()
        pa.release()
        psA.release()

        # ---- Phase B: V projection -> v8 (paired m-planes, fp8 x32) ----
        v_pool = tc.alloc_tile_pool(name="v8", bufs=NT_ALL // 2)
        wV = tc.alloc_tile_pool(name="wV", bufs=2)
        psV = tc.alloc_tile_pool(name="psV", bufs=3, space="PSUM")
        v8 = [v_pool.tile([P, 2, H, HD + 1], F8, tag="v8", name="v8")
              for _ in range(NT_ALL // 2)]
        for grp in range(2):
            wt = wV.tile([P, CJ, 2, 512], F8, tag="wV", name="wV")
            nc.sync.dma_start(out=wt, in_=wv_d[grp])
            for m in range(NT_ALL):
                ps = psV.tile([P, 512], FP, tag="psV", name="psV")
                for j in range(CJ):
                    nc.tensor.matmul(ps, h8[j][:, :, P * m:P * (m + 1)],
                                     wt[:, j, :, :], start=j == 0,
                                     stop=j == CJ - 1, perf_mode=DR)
                nc.scalar.activation(
                    out=v8[m // 2][:, m % 2, 8 * grp:8 * (grp + 1), 0:HD],
                    in_=ps.rearrange("p (h d) -> p h d", h=8), func=AF.Copy,
                    scale=SH / (SH * SW))
        for t in range(NT_ALL // 2):
            nc.vector.memset(v8[t][:, :, :, HD:HD + 1], 1.0)
        wV.release()
        psV.release()

        # right stack: attention outputs (live until proj)
        o8_pool = tc.alloc_tile_pool(name="o8", bufs=CJ, side="right")
        den_pool = tc.alloc_tile_pool(name="den", bufs=2, side="right")
        o8 = [o8_pool.tile([P, 2, N_OWN], F8, tag="o8", name="o8")
              for _ in range(CJ)]

        # prefetched proj weights + residual rows (consumed in Phase D)
        wp_t = wD.tile([P, CJ, 2, C], F8, tag="wD", name="wD")
        nc.sync.dma_start(out=wp_t, in_=wp_d[:])
        xr = [xres.tile([P, C], BF, tag="xres", name="xres")
              for _ in range(NT_OWN)]
        for n in range(NT_OWN):
            nc.gpsimd.dma_start(out=xr[n], in_=xrb_d[P * n:P * (n + 1), :])

        # ---- Phase C: per-head-pair qk + attention ----
        wqk = tc.alloc_tile_pool(name="wqk", bufs=2)
        qT_pool = tc.alloc_tile_pool(name="qT", bufs=2)
        kT_pool = tc.alloc_tile_pool(name="kT", bufs=2)
        pt_pool = tc.alloc_tile_pool(name="pt", bufs=8)
        ot_pool = tc.alloc_tile_pool(name="ot", bufs=2)
        den_row = tc.alloc_tile_pool(name="den_row", bufs=4)
        den_dram = tc.alloc_tile_pool(name="den_dram", bufs=H, space="DRAM")
        psS = tc.alloc_tile_pool(name="psS", bufs=3, space="PSUM")
        psO = tc.alloc_tile_pool(name="psO", bufs=2, space="PSUM")

        for p in range(CT):            # head pairs
            den_p = den_pool.tile([P, N_OWN], FP, tag="den", name="den")
            wt = wqk.tile([P, CJ, 2, 256], F8, tag="wqk", name="wqk")
            nc.sync.dma_start(out=wt, in_=wqk_d[p])
            # pace the first fc1 weight groups through the attention phase
            if p % 3 == 0 and p // 3 < 3:
                nc.sync.dma_start(out=w1_t[p // 3], in_=w1_d[p // 3])
            qTp = qT_pool.tile([P, N_OWN], FPR, tag="qT", name="qT")
            kTp = kT_pool.tile([P, N_ALL], FPR, tag="kT", name="kT")
            otmp = ot_pool.tile([P, N_OWN], FP, tag="ot", name="ot")
            ps = psS.tile([P, 1024], FP, tag="psS", name="psS")
            for j in range(CJ):
                nc.tensor.matmul(ps[:, 0:512], wt[:, j, :, 0:P],
                                 h8[j][:, :, 0:N_OWN], start=j == 0,
                                 stop=j == CJ - 1, perf_mode=DR)
            nc.scalar.activation(out=qTp, in_=ps[:, 0:512], func=AF.Identity,
                                 bias=bq_sb[:, p:p + 1], scale=1.0 / (SH * SWQ))
            for s in range(2):
                ps = psS.tile([P, 1024], FP, tag="psS", name="psS")
                for j in range(CJ):
                    nc.tensor.matmul(ps[:, 512 * s:512 * (s + 1)],
                                     wt[:, j, :, P:256],
                                     h8[j][:, :, 512 * s:512 * (s + 1)],
                                     start=j == 0, stop=j == CJ - 1,
                                     perf_mode=DR)
                nc.scalar.activation(out=kTp[:, 512 * s:512 * (s + 1)],
                                     in_=ps[:, 512 * s:512 * (s + 1)],
                                     func=AF.Copy, scale=1.0 / (SH * SW))

            for odd in range(2):
                h = 2 * p + odd
                kt = kTp[HD * odd:HD * (odd + 1), :]
                qt = qTp[HD * odd:HD * (odd + 1), :]
                pts = []
                for t in range(4):
                    ps = psS.tile([P, 1024], FP, tag="psS", name="psS")
                    nc.tensor.matmul(ps[:, 0:512], kt[:, P * 2 * t:P * (2 * t + 1)],
                                     qt, start=True, stop=True)
                    nc.tensor.matmul(ps[:, 512:1024],
                                     kt[:, P * (2 * t + 1):P * (2 * t + 2)],
                                     qt, start=True, stop=True)
                    pt = pt_pool.tile([P, 2, 512], F8, tag="pt", name="pt")
                    nc.scalar.activation(out=pt, in_=ps.rearrange(
                        "p (a b) -> p a b", a=2), func=AF.Exp, bias=nexp_b,
                        scale=1.0)
                    pts.append(pt)
                po = psO.tile([HD + 1, N_OWN], FP, tag="psO", name="psO")
                for t in range(4):
                    nc.tensor.matmul(po, v8[t][:, :, h, :], pts[t],
                                     start=t == 0, stop=t == 3, perf_mode=DR)
                half = slice(HD * odd, HD * (odd + 1))
                nc.vector.tensor_copy(out=otmp[half, :], in_=po[0:HD, :])
                dr = den_row.tile([1, N_OWN], FP, tag="denrow", name="denrow")
                nc.vector.tensor_copy(out=dr, in_=po[HD:HD + 1, :])
                dd = den_dram.tile([1, N_OWN], FP, tag="dendram", name="dendram")
                nc.gpsimd.dma_start(out=dd, in_=dr)
                nc.gpsimd.dma_start(out=den_p[half, :], in_=_bcast(dd[0, :], HD))
            # finalize the PREVIOUS head pair: by now its den broadcast DMA
            # has landed, so the reciprocal never stalls the vector stream.
            if p > 0:
                fin_den, fin_ot = fin
                for odd in range(2):
                    half = slice(HD * odd, HD * (odd + 1))
                    nc.vector.reciprocal(out=fin_den[half, :],
                                         in_=fin_den[half, :])
                    nc.vector.tensor_mul(
                        out=o8[(p - 1) // 2][half, (p - 1) % 2, :],
                        in0=fin_ot[half, :], in1=fin_den[half, :])
            fin = (den_p, otmp)
        for odd in range(2):
            half = slice(HD * odd, HD * (odd + 1))
            nc.vector.reciprocal(out=fin[0][half, :], in_=fin[0][half, :])
            nc.vector.tensor_mul(out=o8[(CT - 1) // 2][half, (CT - 1) % 2, :],
                                 in0=fin[1][half, :], in1=fin[0][half, :])
        den_row.release()
        ot_pool.release()
        pt_pool.release()
        kT_pool.release()
        qT_pool.release()
        wqk.release()
        v_pool.release()
        h8_pool.release()
        den_dram.release()
        psO.release()
        psS.release()

        # ---- Phase D+E: proj + residual -> x2; LN2 -> h28/r28 (fp8 pairs) --
        pe = tc.alloc_tile_pool(name="pe", bufs=4)
        he = tc.alloc_tile_pool(name="he", bufs=2)
        psD = tc.alloc_tile_pool(name="psD", bufs=4, space="PSUM")
        psE = tc.alloc_tile_pool(name="psE", bufs=4, space="PSUM")
        h28 = [h28_pool.tile([P, 2, N_OWN], F8, tag="h28", name="h28")
               for _ in range(CJ)]
        r28 = [r28_pool.tile([P, 2, N_OWN], F8, tag="r28", name="r28")
               for _ in range(CJ)]
        x2 = [x2_pool.tile([P, C], FP, tag="x2", name="x2")
              for _ in range(NT_OWN)]
        for n in range(NT_OWN):
            pss = [psD.tile([P, 512], FP, tag="psD", name="psD") for _ in range(2)]
            for j in range(CJ):
                for cc in range(2):
                    nc.tensor.matmul(pss[cc], o8[j][:, :, P * n:P * (n + 1)],
                                     wp_t[:, j, :, 512 * cc:512 * (cc + 1)],
                                     start=j == 0, stop=j == CJ - 1,
                                     perf_mode=DR)
            for cc in range(2):
                sl = slice(512 * cc, 512 * (cc + 1))
                nc.vector.scalar_tensor_tensor(
                    out=x2[n][:, sl], in0=pss[cc], scalar=1.0 / (SH * SW),
                    in1=xr[n][:, sl], op0=mybir.AluOpType.mult,
                    op1=mybir.AluOpType.add)
            ht = he.tile([P, C], BF, tag="h2", name="h2")
            _layernorm_tile(nc, pe, ht, x2[n], eps2_sb, 1.0)
            for j in range(CJ):
                ps = psE.tile([P, 2, P], BF, tag="psE", name="psE")
                for t in range(2):
                    nc.tensor.transpose(ps[:, t, :],
                                        ht[:, P * (2 * j + t):P * (2 * j + t + 1)],
                                        identb)
                hdst = h28[j][:, :, P * n:P * (n + 1)]
                if j % 2 == 0:
                    nc.vector.tensor_copy(out=hdst, in_=ps)
                else:
                    nc.scalar.activation(out=hdst, in_=ps, func=AF.Copy)
                nc.vector.scalar_tensor_tensor(
                    out=r28[j][:, :, P * n:P * (n + 1)], in0=ps, scalar=1.0,
                    in1=hdst, op0=mybir.AluOpType.mult,
                    op1=mybir.AluOpType.subtract)
        he.release()
        pe.release()
        xres.release()
        wD.release()
        den_pool.release()
        o8_pool.release()
        psE.release()
        psD.release()

        # ---- Phase F: fc1 (3-set fp8 DR) + gelu -> h38/r38 fp8 pairs ----
        h38_pool = tc.alloc_tile_pool(name="h38", bufs=HID // 256, side="right")
        r38_pool = tc.alloc_tile_pool(name="r38", bufs=HID // 256, side="right")
        hb = tc.alloc_tile_pool(name="hb", bufs=3)
        psF = tc.alloc_tile_pool(name="psF", bufs=3, space="PSUM")
        h38 = [h38_pool.tile([P, 2, N_OWN], F8, tag="h38", name="h38")
               for _ in range(HID // 256)]
        r38 = [r38_pool.tile([P, 2, N_OWN], F8, tag="r38", name="r38")
               for _ in range(HID // 256)]
        for g in range(8):             # groups of 4 hf-tiles
            if g + 3 < 8:              # stream the next fc1 weight group
                nc.sync.dma_start(out=w1_t[g + 3], in_=w1_d[g + 3])
            if g < 4:                  # pace the fc2 weight stream through fc1
                nc.sync.dma_start(out=wf2_t[g], in_=wf2_d[g])
            for f in range(4):
                hf = 4 * g + f
                ps = psF.tile([P, 512], FP, tag="psF", name="psF")
                k = 0
                for rhs_set, hl in ((h28, 0), (h28, 1), (r28, 0)):
                    for j in range(CJ):
                        nc.tensor.matmul(
                            ps, w1_t[g][:, hl, j, :, P * f:P * (f + 1)],
                            rhs_set[j], start=k == 0, stop=k == 3 * CJ - 1,
                            perf_mode=DR)
                        k += 1
                gb = hb.tile([P, N_OWN], BF, tag="gb", name="gb")
                nc.scalar.activation(out=gb, in_=ps, func=AF.Gelu,
                                     bias=b2_sb[:, hf:hf + 1], scale=1.0 / SW)
                gdst = h38[hf // 2][:, hf % 2, :]
                nc.scalar.activation(out=gdst, in_=gb, func=AF.Copy)
                nc.vector.scalar_tensor_tensor(
                    out=r38[hf // 2][:, hf % 2, :], in0=gb, scalar=1.0,
                    in1=gdst, op0=mybir.AluOpType.mult,
                    op1=mybir.AluOpType.subtract)
        psF.release()
        hb.release()
        wF.release()
        r28_pool.release()
        h28_pool.release()

        # ---- Phase G: fc2 (3-set fp8 DR) + residual -> out, n-outer ----
        psG = tc.alloc_tile_pool(name="psG", bufs=2, space="PSUM")
        out_pool = tc.alloc_tile_pool(name="outp", bufs=2)
        psB = psG.tile([P, C], FP, tag="psB", name="psB")
        for cc in range(2):
            nc.tensor.matmul(psB[:, 512 * cc:512 * (cc + 1)], ones_row,
                             bf2_row[:, 512 * cc:512 * (cc + 1)],
                             start=True, stop=True)
        NJ2 = HID // 256               # 16 hid pair-chunks
        for n in range(NT_OWN):
            pg = [psG.tile([P, 512], FP, tag=f"psG{cc}", name="psG")
                  for cc in range(2)]
            for cc in range(2):
                k = 0
                for lhs_set, hl in ((h38, 0), (h38, 1), (r38, 0)):
                    for j in range(NJ2):
                        nc.tensor.matmul(
                            pg[cc], lhs_set[j][:, :, P * n:P * (n + 1)],
                            wf2_t[j // 4][:, j % 4, hl, :,
                                          512 * cc:512 * (cc + 1)],
                            start=k == 0, stop=k == 3 * NJ2 - 1,
                            perf_mode=DR)
                        k += 1
            x3 = out_pool.tile([P, C], FP, tag="x3", name="x3")
            for cc in range(2):
                sl = slice(512 * cc, 512 * (cc + 1))
                nc.vector.scalar_tensor_tensor(
                    out=x3[:, sl], in0=pg[cc], scalar=1.0 / (2 * SW),
                    in1=x2[n][:, sl], op0=mybir.AluOpType.mult,
                    op1=mybir.AluOpType.add)
                nc.vector.tensor_add(out=x3[:, sl], in0=x3[:, sl],
                                     in1=psB[:, sl])
            nc.gpsimd.dma_start(out=out_d[P * n:P * (n + 1), :], in_=x3)
        out_pool.release()
        r38_pool.release()
        h38_pool.release()
        x2_pool.release()
        wf2_pool.release()
        psG.release()
        consts.release()

    nc.compile()
    return nc


_NC = None


def _get_nc():
    global _NC
    if _NC is None:
        _NC = build()
    return _NC


def _q8(a, s):
    return np.clip(np.asarray(a, np.float32) * s, -240.0, 240.0).astype(NP_F8)


def _prep(inputs):
    f32 = lambda a: np.ascontiguousarray(np.asarray(a, dtype=np.float32))
    x = f32(inputs["x"])
    qkv_w, qkv_b = f32(inputs["qkv_w"]), f32(inputs["qkv_b"])
    proj_w, proj_b = f32(inputs["proj_w"]), f32(inputs["proj_b"])
    fc1_w, fc1_b = f32(inputs["fc1_w"]), f32(inputs["fc1_b"])
    fc2_w, fc2_b = f32(inputs["fc2_w"]), f32(inputs["fc2_b"])
    ln1_g, ln1_b = f32(inputs["ln1_g"]), f32(inputs["ln1_b"])
    ln2_g, ln2_b = f32(inputs["ln2_g"]), f32(inputs["ln2_b"])

    scale = np.float32(HD ** -0.5)
    w1 = (qkv_w * ln1_g[None, :]).T                 # [C, 3C]
    b1 = qkv_b + qkv_w @ ln1_b                      # [3C]
    wq = w1[:, :C] * scale
    wk = w1[:, C:2 * C]
    wv = w1[:, 2 * C:]
    # DoubleRow plane-pair packing: [p, kp, j, t, m] = w[(2j+t)*128+kp, col]
    def pack(w, s):
        # w: [C, M] fp32 -> [128, CJ, 2, M] fp8 with k-pairs in planes
        wq_ = _q8(w, s)                             # [C, M]
        return np.ascontiguousarray(
            wq_.reshape(CJ, 2, P, -1).transpose(2, 0, 1, 3))
    wqk8 = np.empty((CT, P, CJ, 2, 256), dtype=NP_F8)
    for p_ in range(CT):
        wqk8[p_, :, :, :, 0:P] = pack(wq[:, P * p_:P * (p_ + 1)], SWQ)
        wqk8[p_, :, :, :, P:256] = pack(wk[:, P * p_:P * (p_ + 1)], SW)
    wv8 = np.empty((2, P, CJ, 2, 512), dtype=NP_F8)
    for g in range(2):
        wv8[g] = pack(wv[:, 512 * g:512 * (g + 1)], SW)
    bq = (b1[:C] * scale).copy()
    bv = b1[2 * C:]
    wp8 = pack(proj_w.T, SW)                        # [128, CJ, 2, C]
    bp = proj_b + proj_w @ bv
    def hilo(w, s):
        hi = _q8(w, s)                              # [K, M] fp8
        lo = _q8(w - hi.astype(np.float32) / s, s)
        return hi, lo

    w1w = (fc1_w * ln2_g[None, :]).T                # [C, HID]
    w1hi, w1lo = hilo(w1w, SW)
    # [8, P, 2, CJ, 2, 512]: g, partition, hi/lo, k-pair, k-plane, hid
    def pack_dr(wq_):                               # [C, M] fp8 -> plane pairs
        return wq_.reshape(CJ, 2, P, -1).transpose(2, 0, 1, 3)
    w1c = np.empty((8, P, 2, CJ, 2, 512), dtype=NP_F8)
    for g in range(8):
        w1c[g, :, 0] = pack_dr(w1hi[:, 512 * g:512 * (g + 1)])
        w1c[g, :, 1] = pack_dr(w1lo[:, 512 * g:512 * (g + 1)])
    b2 = fc1_b + fc1_w @ ln2_b
    wf2T = fc2_w.T                                  # [HID, C]
    w2hi, w2lo = hilo(wf2T, 2 * SW)
    # [4, P, 4, 2, 2, C]: chunk, partition, pair-in-chunk, hi/lo, k-plane, out
    wf2c = np.empty((4, P, 4, 2, 2, C), dtype=NP_F8)
    for hl, w_ in ((0, w2hi), (1, w2lo)):
        wf2c[:, :, :, hl] = w_.reshape(4, 4, 2, P, C).transpose(0, 3, 1, 2, 4)
    bf2 = fc2_b

    shared = dict(wqk=wqk8, wv=wv8,
                  bq=f32(bq.reshape(CT, P).T), wp=wp8,
                  w1=w1c, b2=f32(b2.reshape(HID // P, P).T),
                  wf2=wf2c, bf2=f32(bf2))
    in_maps = []
    for c in range(N_CORES):
        b, half = divmod(c, 2)
        own = x[b, N_OWN * half:N_OWN * (half + 1), :]
        oth = x[b, N_OWN * (1 - half):N_OWN * (2 - half), :]
        xp = np.concatenate([own, oth], axis=0)
        xrb = own + bp[None, :]
        in_maps.append({"x": xp.astype(NP_BF), "xrb": xrb.astype(NP_BF),
                        **shared})
    return in_maps


def run(inputs, trace=False, trace_kwargs=None):
    from concourse.bass_utils import run_bass_kernel_spmd
    nc = _get_nc()
    in_maps = _prep(inputs)
    res = run_bass_kernel_spmd(nc, in_maps, core_ids=list(range(N_CORES)),
                               trace=trace, **(trace_kwargs or {}))
    B = 4
    out = np.empty((B, N_ALL, C), dtype=np.float32)
    for c in range(N_CORES):
        b, half = divmod(c, 2)
        out[b, N_OWN * half:N_OWN * (half + 1), :] = res.results[c]["out"]
    return out, res


def kernel(**inputs):
    out, _ = run(inputs, trace=False)
    return out


# revision 70
# speedup vs baseline: 1.1198x; 1.1198x over previous
"""Dense transformer block on 8 Trainium2 NeuronCores — fp8 DoubleRow edition.

Sharding: each core owns half a batch element (512 rows out of [4, 1024, C]).
Cores redundantly compute LN1 + K/V projections for the full batch element
(so attention needs no cross-core communication); Q / attention / proj / MLP
run only on the core's own 512 rows.  No collectives.

Precision plan (rel-l2 budget 2e-2; this lands ~6e-3):
  - x streamed as bf16 for the LN1 path; fp32 rows reloaded for the residual.
  - attention-side matmuls (q/k/v proj, attn@V, proj) in fp8e4m3 DoubleRow
    (2 k-tiles per pass, 0.5 PE cycles/row): activations scaled x32, weights
    x1024 (q-weights x8192 with softmax scale folded), compensated in the
    PSUM->SBUF copies.  q/k kept fp32r; scores matmul fp32r; exp output
    quantized to fp8 (exp(s)/4 to dodge e4m3 saturation at 240) — softmax
    renormalization by the appended ones-column cancels P-quantization scale.
  - MLP (fc1/gelu/fc2) in bf16 end-to-end: same PE rate as fp32r, half DMA.

Host-side prep:
  - weights pre-transposed to [in_feat, out_feat], LN gammas folded into the
    following matmul weights, betas into biases; q scale folded into W_q/b_q;
    k bias dropped (softmax shift-invariant), v bias folded into proj bias;
    proj bias folded into the residual rows (xrb = x_own + bp).
  - fp8 weights packed in DoubleRow plane-pair layout [128, k-pair, 2, out].
  - per-core x rows permuted to [own 512 | other 512]; softmax is invariant
    to key/value ordering so attention over permuted K/V is exact.
"""

import sys

if "/opt/trn_rl_repo" not in sys.path:
    sys.path.insert(0, "/opt/trn_rl_repo")

import numpy as np
import ml_dtypes

import concourse.bacc as bacc
import concourse.bass as bass
import concourse.mybir as mybir
import concourse.tile as tile
from concourse.masks import make_identity

FP = mybir.dt.float32
FPR = mybir.dt.float32r
BF = mybir.dt.bfloat16
F8 = mybir.dt.float8e4
AF = mybir.ActivationFunctionType
DR = mybir.MatmulPerfMode.DoubleRow

NP_BF = ml_dtypes.bfloat16
NP_F8 = ml_dtypes.float8_e4m3

N_CORES = 8
P = 128
C = 1024            # model dim
H = 16              # heads
HD = 64             # head dim
HID = 4096          # mlp hidden
N_ALL = 1024        # rows per batch element
N_OWN = 512         # rows owned per core
EPS = 1e-5

CT = C // P         # 8 feature chunks of 128
CJ = CT // 2        # 4 DoubleRow plane-pairs
NT_ALL = N_ALL // P # 8 row tiles
NT_OWN = N_OWN // P # 4 row tiles

# quantization scales (powers of two; compensated in PSUM->SBUF copies)
SH = 32.0           # fp8 activation scale (h, v, o)
SW = 1024.0         # fp8 weight scale (wk, wv, wp)
SWQ = 8192.0        # fp8 q-weight scale (softmax 1/8 folded in first)
LN2_2 = float(2.0 * np.log(2.0))


def _bcast(ap, p=P):
    """Partition-broadcast a [*free] AP to [p, *free] (step-0 partition dim)."""
    return bass.AP(tensor=ap.tensor, offset=ap.offset, ap=[[0, p], *ap.ap])


def _layernorm_tile(nc, pool, out, xt, eps_sb, sqrt_scale, ts_eng=None):
    """out = (xt - mean) * (1/sqrt_scale) / sqrt(var + eps), rowwise over C.

    eps_sb must hold EPS * sqrt_scale^2 so that
    rstd = 1 / sqrt(sqrt_scale^2 * var + eps * sqrt_scale^2)
         = (1/sqrt_scale) / sqrt(var + eps).
    Used with sqrt_scale=2^-5 to get an extra x32 folded into the output.
    """
    st = pool.tile([P, 2, 6], FP, tag="ln_st", name="ln_st")
    for g in range(2):
        nc.vector.bn_stats(out=st[:, g, :], in_=xt[:, 512 * g:512 * (g + 1)])
    mv = pool.tile([P, 2], FP, tag="ln_mv", name="ln_mv")
    nc.vector.bn_aggr(out=mv, in_=st)
    rstd = pool.tile([P, 1], FP, tag="ln_rstd", name="ln_rstd")
    nc.scalar.activation(out=rstd, in_=mv[:, 1:2], func=AF.Sqrt, bias=eps_sb,
                         scale=sqrt_scale * sqrt_scale)
    nc.vector.reciprocal(out=rstd, in_=rstd)
    ts = ts_eng if ts_eng is not None else nc.vector
    for g in range(2):
        sl = slice(512 * g, 512 * (g + 1))
        ts.tensor_scalar(
            out=out[:, sl], in0=xt[:, sl],
            scalar1=mv[:, 0:1], scalar2=rstd,
            op0=mybir.AluOpType.subtract, op1=mybir.AluOpType.mult,
        )


def build():
    nc = bacc.Bacc("TRN2", target_bir_lowering=False, debug=False,
                   num_devices=N_CORES)

    x_d = nc.dram_tensor("x", [N_ALL, C], BF, kind="ExternalInput")
    xrb_d = nc.dram_tensor("xrb", [N_OWN, C], BF, kind="ExternalInput")
    wqk_d = nc.dram_tensor("wqk", [CT, P, CJ, 2, 256], F8, kind="ExternalInput")
    wv_d = nc.dram_tensor("wv", [2, P, CJ, 2, 512], F8, kind="ExternalInput")
    bq_d = nc.dram_tensor("bq", [P, CT], FP, kind="ExternalInput")
    wp_d = nc.dram_tensor("wp", [P, CJ, 2, C], F8, kind="ExternalInput")
    w1_d = nc.dram_tensor("w1", [8, P, CT, 512], BF, kind="ExternalInput")
    b2_d = nc.dram_tensor("b2", [P, HID // P], FP, kind="ExternalInput")
    wf2_d = nc.dram_tensor("wf2", [4, P, 8, C], BF, kind="ExternalInput")
    bf2_d = nc.dram_tensor("bf2", [C], FP, kind="ExternalInput")
    out_d = nc.dram_tensor("out", [N_OWN, C], FP, kind="ExternalOutput")

    with tile.TileContext(nc, pool_alloc_mode="queue") as tc:
        consts = tc.alloc_tile_pool(name="consts", bufs=1)
        identb = consts.tile([P, P], BF)
        make_identity(nc, identb)
        bq_sb = consts.tile([P, CT], FP)
        nc.sync.dma_start(out=bq_sb, in_=bq_d[:])
        b2_sb = consts.tile([P, HID // P], FP)
        nc.sync.dma_start(out=b2_sb, in_=b2_d[:])
        bf2_fp = consts.tile([1, C], FP)
        nc.sync.dma_start(out=bf2_fp, in_=bf2_d[:].rearrange("(o c) -> o c", o=1))
        bf2_row = consts.tile([1, C], FPR)      # PE-broadcast in Phase G
        nc.vector.tensor_copy(out=bf2_row, in_=bf2_fp)
        eps1_sb = consts.tile([P, 1], FP)       # for LN1: eps * (2^-5)^2
        nc.vector.memset(eps1_sb, EPS / (SH * SH))
        eps2_sb = consts.tile([P, 1], FP)       # for LN2: plain eps
        nc.vector.memset(eps2_sb, EPS)
        nexp_b = consts.tile([P, 1], FP)        # exp bias: -2ln2 -> exp(s)/4
        nc.vector.memset(nexp_b, -LN2_2)
        ones_row = consts.tile([1, P], FPR)
        nc.vector.memset(ones_row.bitcast(FP), 1.0)

        # fc2 weights (8 hf-chunks per tile): fully resident; DMA triggers
        # paced through the fc1 loop.
        wf2_pool = tc.alloc_tile_pool(name="wf2", bufs=4, side="right")
        wf2_t = [wf2_pool.tile([P, 8, C], BF, tag="wf2", name="wf2")
                 for _ in range(4)]

        # Long-lived pools allocated up-front in reverse order of death so
        # every release happens at the top of the LIFO pool stack:
        # x2 dies at phase G end, h2T/wF at F end, wD/xres at D end.
        x2_pool = tc.alloc_tile_pool(name="x2", bufs=NT_OWN)
        h2T_pool = tc.alloc_tile_pool(name="h2T", bufs=CJ)
        wF = tc.alloc_tile_pool(name="wF", bufs=3)
        wD = tc.alloc_tile_pool(name="wD", bufs=1)
        xres = tc.alloc_tile_pool(name="xres", bufs=NT_OWN)
        # fc1 weights: ring of 3 group-tiles; first 3 DMAs paced through the
        # attention loop, the rest triggered as fc1 consumes groups.
        w1_t = [wF.tile([P, CT, 512], BF, tag="wF", name="wF")
                for _ in range(8)]

        # ---- Phase A: load x (bf16), LN1 -> h fp8 (x32), transpose ----
        h8_pool = tc.alloc_tile_pool(name="h8", bufs=CJ)
        pa = tc.alloc_tile_pool(name="pa", bufs=4)
        ha = tc.alloc_tile_pool(name="ha", bufs=2)
        psA = tc.alloc_tile_pool(name="psA", bufs=6, space="PSUM")
        h8 = [h8_pool.tile([P, 2, N_ALL], F8, tag="h8", name="h8")
              for _ in range(CJ)]
        x_tiles = []
        for i in range(NT_ALL):
            xt = pa.tile([P, C], BF, tag="x_in", name="x_in")
            nc.gpsimd.dma_start(out=xt, in_=x_d[P * i:P * (i + 1), :])
            x_tiles.append(xt)
        for i in range(NT_ALL):
            ht = ha.tile([P, C], BF, tag="h", name="h")
            _layernorm_tile(nc, pa, ht, x_tiles[i], eps1_sb, 1.0 / SH)
            for j in range(CJ):       # transpose chunk pairs, copy as one
                ps = psA.tile([P, 2, P], BF, tag="psA", name="psA")
                for t in range(2):
                    nc.tensor.transpose(ps[:, t, :],
                                        ht[:, P * (2 * j + t):P * (2 * j + t + 1)],
                                        identb)
                dst = h8[j][:, :, P * i:P * (i + 1)]
                if j % 2 == 0:
                    nc.vector.tensor_copy(out=dst, in_=ps)
                else:
                    nc.scalar.activation(out=dst, in_=ps, func=AF.Copy)
        ha.release()
        pa.release()
        psA.release()

        # ---- Phase B: V projection -> v8 (paired m-planes, fp8 x32) ----
        v_pool = tc.alloc_tile_pool(name="v8", bufs=NT_ALL // 2)
        wV = tc.alloc_tile_pool(name="wV", bufs=2)
        psV = tc.alloc_tile_pool(name="psV", bufs=3, space="PSUM")
        v8 = [v_pool.tile([P, 2, H, HD + 1], F8, tag="v8", name="v8")
              for _ in range(NT_ALL // 2)]
        for grp in range(2):
            wt = wV.tile([P, CJ, 2, 512], F8, tag="wV", name="wV")
            nc.sync.dma_start(out=wt, in_=wv_d[grp])
            for m0 in range(0, NT_ALL, 2):   # interleave two m accumulation
                pss = [psV.tile([P, 512], FP, tag="psV", name="psV")
                       for _ in range(2)]
                for j in range(CJ):
                    for mm_ in range(2):
                        nc.tensor.matmul(
                            pss[mm_], h8[j][:, :, P * (m0 + mm_):P * (m0 + mm_ + 1)],
                            wt[:, j, :, :], start=j == 0,
                            stop=j == CJ - 1, perf_mode=DR)
                for mm_ in range(2):
                    m = m0 + mm_
                    nc.scalar.activation(
                        out=v8[m // 2][:, m % 2, 8 * grp:8 * (grp + 1), 0:HD],
                        in_=pss[mm_].rearrange("p (h d) -> p h d", h=8),
                        func=AF.Copy, scale=SH / (SH * SW))
        for t in range(NT_ALL // 2):
            nc.vector.memset(v8[t][:, :, :, HD:HD + 1], 1.0)
        wV.release()
        psV.release()

        # right stack: attention outputs (live until proj)
        o8_pool = tc.alloc_tile_pool(name="o8", bufs=CJ, side="right")
        den_pool = tc.alloc_tile_pool(name="den", bufs=2, side="right")
        o8 = [o8_pool.tile([P, 2, N_OWN], F8, tag="o8", name="o8")
              for _ in range(CJ)]

        # prefetched proj weights + residual rows (consumed in Phase D)
        wp_t = wD.tile([P, CJ, 2, C], F8, tag="wD", name="wD")
        nc.sync.dma_start(out=wp_t, in_=wp_d[:])
        xr = [xres.tile([P, C], BF, tag="xres", name="xres")
              for _ in range(NT_OWN)]
        for n in range(NT_OWN):
            nc.gpsimd.dma_start(out=xr[n], in_=xrb_d[P * n:P * (n + 1), :])

        # ---- Phase C: per-head-pair qk + attention ----
        wqk = tc.alloc_tile_pool(name="wqk", bufs=2)
        qT_pool = tc.alloc_tile_pool(name="qT", bufs=2)
        kT_pool = tc.alloc_tile_pool(name="kT", bufs=2)
        pt_pool = tc.alloc_tile_pool(name="pt", bufs=8)
        ot_pool = tc.alloc_tile_pool(name="ot", bufs=2)
        den_row = tc.alloc_tile_pool(name="den_row", bufs=4)
        den_dram = tc.alloc_tile_pool(name="den_dram", bufs=H, space="DRAM")
        psS = tc.alloc_tile_pool(name="psS", bufs=3, space="PSUM")
        psO = tc.alloc_tile_pool(name="psO", bufs=2, space="PSUM")

        for p in range(CT):            # head pairs
            den_p = den_pool.tile([P, N_OWN], FP, tag="den", name="den")
            wt = wqk.tile([P, CJ, 2, 256], F8, tag="wqk", name="wqk")
            nc.sync.dma_start(out=wt, in_=wqk_d[p])
            # pace the first fc1 weight groups through the attention phase
            if p % 3 == 0 and p // 3 < 3:
                nc.sync.dma_start(out=w1_t[p // 3], in_=w1_d[p // 3])
            qTp = qT_pool.tile([P, N_OWN], FPR, tag="qT", name="qT")
            kTp = kT_pool.tile([P, N_ALL], FPR, tag="kT", name="kT")
            otmp = ot_pool.tile([P, N_OWN], FP, tag="ot", name="ot")
            # three accumulation chains (q, k-half0, k-half1) interleaved so
            # each LDWEIGHTS overlaps another chain's stream
            psq = psS.tile([P, 1024], FP, tag="psS", name="psS")
            psk = [psS.tile([P, 1024], FP, tag="psS", name="psS")
                   for _ in range(2)]
            for j in range(CJ):
                nc.tensor.matmul(psq[:, 0:512], wt[:, j, :, 0:P],
                                 h8[j][:, :, 0:N_OWN], start=j == 0,
                                 stop=j == CJ - 1, perf_mode=DR)
                for s in range(2):
                    nc.tensor.matmul(psk[s][:, 512 * s:512 * (s + 1)],
                                     wt[:, j, :, P:256],
                                     h8[j][:, :, 512 * s:512 * (s + 1)],
                                     start=j == 0, stop=j == CJ - 1,
                                     perf_mode=DR)
            nc.scalar.activation(out=qTp, in_=psq[:, 0:512], func=AF.Identity,
                                 bias=bq_sb[:, p:p + 1], scale=1.0 / (SH * SWQ))
            for s in range(2):
                nc.scalar.activation(out=kTp[:, 512 * s:512 * (s + 1)],
                                     in_=psk[s][:, 512 * s:512 * (s + 1)],
                                     func=AF.Copy, scale=1.0 / (SH * SW))

            pts_all = []
            for odd in range(2):
                kt = kTp[HD * odd:HD * (odd + 1), :]
                qt = qTp[HD * odd:HD * (odd + 1), :]
                pts = []
                for t in range(4):
                    ps = psS.tile([P, 1024], FP, tag="psS", name="psS")
                    nc.tensor.matmul(ps[:, 0:512], kt[:, P * 2 * t:P * (2 * t + 1)],
                                     qt, start=True, stop=True)
                    nc.tensor.matmul(ps[:, 512:1024],
                                     kt[:, P * (2 * t + 1):P * (2 * t + 2)],
                                     qt, start=True, stop=True)
                    pt = pt_pool.tile([P, 2, 512], F8, tag="pt", name="pt")
                    nc.scalar.activation(out=pt, in_=ps.rearrange(
                        "p (a b) -> p a b", a=2), func=AF.Exp, bias=nexp_b,
                        scale=1.0)
                    pts.append(pt)
                pts_all.append(pts)
            pos = [psO.tile([HD + 1, N_OWN], FP, tag="psO", name="psO")
                   for _ in range(2)]
            for t in range(4):       # two interleaved AV accumulation chains
                for odd in range(2):
                    nc.tensor.matmul(pos[odd], v8[t][:, :, 2 * p + odd, :],
                                     pts_all[odd][t], start=t == 0,
                                     stop=t == 3, perf_mode=DR)
            for odd in range(2):
                po = pos[odd]
                half = slice(HD * odd, HD * (odd + 1))
                nc.vector.tensor_copy(out=otmp[half, :], in_=po[0:HD, :])
                dr = den_row.tile([1, N_OWN], FP, tag="denrow", name="denrow")
                nc.vector.tensor_copy(out=dr, in_=po[HD:HD + 1, :])
                dd = den_dram.tile([1, N_OWN], FP, tag="dendram", name="dendram")
                nc.gpsimd.dma_start(out=dd, in_=dr)
                nc.gpsimd.dma_start(out=den_p[half, :], in_=_bcast(dd[0, :], HD))
            # finalize the PREVIOUS head pair: by now its den broadcast DMA
            # has landed, so the reciprocal never stalls the vector stream.
            if p > 0:
                fin_den, fin_ot = fin
                for odd in range(2):
                    half = slice(HD * odd, HD * (odd + 1))
                    nc.vector.reciprocal(out=fin_den[half, :],
                                         in_=fin_den[half, :])
                    nc.vector.tensor_mul(
                        out=o8[(p - 1) // 2][half, (p - 1) % 2, :],
                        in0=fin_ot[half, :], in1=fin_den[half, :])
            fin = (den_p, otmp)
        for odd in range(2):
            half = slice(HD * odd, HD * (odd + 1))
            nc.vector.reciprocal(out=fin[0][half, :], in_=fin[0][half, :])
            nc.vector.tensor_mul(out=o8[(CT - 1) // 2][half, (CT - 1) % 2, :],
                                 in0=fin[1][half, :], in1=fin[0][half, :])
        den_row.release()
        ot_pool.release()
        pt_pool.release()
        kT_pool.release()
        qT_pool.release()
        wqk.release()
        v_pool.release()
        h8_pool.release()
        den_dram.release()
        psO.release()
        psS.release()

        # ---- Phase D+E: proj + residual -> x2; LN2 -> h2T (bf16 pairs) ----
        pe = tc.alloc_tile_pool(name="pe", bufs=4)
        he = tc.alloc_tile_pool(name="he", bufs=2)
        psD = tc.alloc_tile_pool(name="psD", bufs=4, space="PSUM")
        psE = tc.alloc_tile_pool(name="psE", bufs=4, space="PSUM")
        h2T = [h2T_pool.tile([P, 2, N_OWN], BF, tag="h2T", name="h2T")
               for _ in range(CJ)]
        x2 = [x2_pool.tile([P, C], FP, tag="x2", name="x2")
              for _ in range(NT_OWN)]
        for n in range(NT_OWN):
            pss = [psD.tile([P, 512], FP, tag="psD", name="psD") for _ in range(2)]
            for j in range(CJ):
                for cc in range(2):
                    nc.tensor.matmul(pss[cc], o8[j][:, :, P * n:P * (n + 1)],
                                     wp_t[:, j, :, 512 * cc:512 * (cc + 1)],
                                     start=j == 0, stop=j == CJ - 1,
                                     perf_mode=DR)
            for cc in range(2):
                sl = slice(512 * cc, 512 * (cc + 1))
                nc.vector.scalar_tensor_tensor(
                    out=x2[n][:, sl], in0=pss[cc], scalar=1.0 / (SH * SW),
                    in1=xr[n][:, sl], op0=mybir.AluOpType.mult,
                    op1=mybir.AluOpType.add)
            ht = he.tile([P, C], BF, tag="h2", name="h2")
            _layernorm_tile(nc, pe, ht, x2[n], eps2_sb, 1.0)
            for j in range(CJ):
                ps = psE.tile([P, 2, P], BF, tag="psE", name="psE")
                for t in range(2):
                    nc.tensor.transpose(ps[:, t, :],
                                        ht[:, P * (2 * j + t):P * (2 * j + t + 1)],
                                        identb)
                hdst = h2T[j][:, :, P * n:P * (n + 1)]
                if j % 2 == 0:
                    nc.vector.tensor_copy(out=hdst, in_=ps)
                else:
                    nc.scalar.activation(out=hdst, in_=ps, func=AF.Copy)
        he.release()
        pe.release()
        xres.release()
        wD.release()
        den_pool.release()
        o8_pool.release()
        psE.release()
        psD.release()

        # ---- Phase F: fc1 + gelu -> h3T bf16 [HID, N_OWN] ----
        h3T_pool = tc.alloc_tile_pool(name="h3T", bufs=HID // P, side="right")
        psF = tc.alloc_tile_pool(name="psF", bufs=3, space="PSUM")
        h3T = [h3T_pool.tile([P, N_OWN], BF, tag="h3T", name="h3T")
               for _ in range(HID // P)]
        for g in range(8):             # groups of 4 hf-tiles
            if g + 3 < 8:              # stream the next fc1 weight group
                nc.sync.dma_start(out=w1_t[g + 3], in_=w1_d[g + 3])
            if g < 4:                  # pace the fc2 weight stream through fc1
                nc.sync.dma_start(out=wf2_t[g], in_=wf2_d[g])
            for f in range(4):
                hf = 4 * g + f
                ps = psF.tile([P, 512], FP, tag="psF", name="psF")
                for c in range(CT):
                    nc.tensor.matmul(ps, w1_t[g][:, c, P * f:P * (f + 1)],
                                     h2T[c // 2][:, c % 2, :],
                                     start=c == 0, stop=c == CT - 1)
                nc.scalar.activation(out=h3T[hf], in_=ps, func=AF.Gelu,
                                     bias=b2_sb[:, hf:hf + 1], scale=1.0)
        psF.release()
        wF.release()
        h2T_pool.release()

        # ---- Phase G: fc2 + residual -> out (n-outer so tiles finish early) --
        psG = tc.alloc_tile_pool(name="psG", bufs=2, space="PSUM")
        out_pool = tc.alloc_tile_pool(name="outp", bufs=2)
        psB = psG.tile([P, C], FP, tag="psB", name="psB")
        for cc in range(2):
            nc.tensor.matmul(psB[:, 512 * cc:512 * (cc + 1)], ones_row,
                             bf2_row[:, 512 * cc:512 * (cc + 1)],
                             start=True, stop=True)
        for n in range(NT_OWN):
            pg = [psG.tile([P, 512], FP, tag=f"psG{cc}", name="psG")
                  for cc in range(2)]
            for hf in range(HID // P):
                for cc in range(2):
                    nc.tensor.matmul(pg[cc],
                                     h3T[hf][:, P * n:P * (n + 1)],
                                     wf2_t[hf // 8][:, hf % 8,
                                                    512 * cc:512 * (cc + 1)],
                                     start=hf == 0, stop=hf == HID // P - 1)
            x3 = out_pool.tile([P, C], FP, tag="x3", name="x3")
            for cc in range(2):
                sl = slice(512 * cc, 512 * (cc + 1))
                nc.vector.scalar_tensor_tensor(
                    out=x3[:, sl], in0=pg[cc], scalar=1.0,
                    in1=x2[n][:, sl], op0=mybir.AluOpType.mult,
                    op1=mybir.AluOpType.add)
                nc.vector.tensor_add(out=x3[:, sl], in0=x3[:, sl],
                                     in1=psB[:, sl])
            nc.gpsimd.dma_start(out=out_d[P * n:P * (n + 1), :], in_=x3)
        out_pool.release()
        h3T_pool.release()
        x2_pool.release()
        wf2_pool.release()
        psG.release()
        consts.release()

    nc.compile()
    return nc


_NC = None


def _get_nc():
    global _NC
    if _NC is None:
        _NC = build()
    return _NC


def _q8(a, s):
    return np.clip(np.asarray(a, np.float32) * s, -240.0, 240.0).astype(NP_F8)


def _prep(inputs):
    f32 = lambda a: np.ascontiguousarray(np.asarray(a, dtype=np.float32))
    x = f32(inputs["x"])
    qkv_w, qkv_b = f32(inputs["qkv_w"]), f32(inputs["qkv_b"])
    proj_w, proj_b = f32(inputs["proj_w"]), f32(inputs["proj_b"])
    fc1_w, fc1_b = f32(inputs["fc1_w"]), f32(inputs["fc1_b"])
    fc2_w, fc2_b = f32(inputs["fc2_w"]), f32(inputs["fc2_b"])
    ln1_g, ln1_b = f32(inputs["ln1_g"]), f32(inputs["ln1_b"])
    ln2_g, ln2_b = f32(inputs["ln2_g"]), f32(inputs["ln2_b"])

    scale = np.float32(HD ** -0.5)
    w1 = (qkv_w * ln1_g[None, :]).T                 # [C, 3C]
    b1 = qkv_b + qkv_w @ ln1_b                      # [3C]
    wq = w1[:, :C] * scale
    wk = w1[:, C:2 * C]
    wv = w1[:, 2 * C:]
    # DoubleRow plane-pair packing: [p, kp, j, t, m] = w[(2j+t)*128+kp, col]
    def pack(w, s):
        # w: [C, M] fp32 -> [128, CJ, 2, M] fp8 with k-pairs in planes
        wq_ = _q8(w, s)                             # [C, M]
        return np.ascontiguousarray(
            wq_.reshape(CJ, 2, P, -1).transpose(2, 0, 1, 3))
    wqk8 = np.empty((CT, P, CJ, 2, 256), dtype=NP_F8)
    for p_ in range(CT):
        wqk8[p_, :, :, :, 0:P] = pack(wq[:, P * p_:P * (p_ + 1)], SWQ)
        wqk8[p_, :, :, :, P:256] = pack(wk[:, P * p_:P * (p_ + 1)], SW)
    wv8 = np.empty((2, P, CJ, 2, 512), dtype=NP_F8)
    for g in range(2):
        wv8[g] = pack(wv[:, 512 * g:512 * (g + 1)], SW)
    bq = (b1[:C] * scale).copy()
    bv = b1[2 * C:]
    wp8 = pack(proj_w.T, SW)                        # [128, CJ, 2, C]
    bp = proj_b + proj_w @ bv
    w1w = (fc1_w * ln2_g[None, :]).T                # [C, HID]
    w1b = np.ascontiguousarray(
        w1w.astype(NP_BF).reshape(CT, P, 8, 512).transpose(2, 1, 0, 3))
    b2 = fc1_b + fc1_w @ ln2_b
    wf2b = np.ascontiguousarray(
        fc2_w.T.astype(NP_BF).reshape(4, 8, P, C).transpose(0, 2, 1, 3))
    bf2 = fc2_b

    shared = dict(wqk=wqk8, wv=wv8,
                  bq=f32(bq.reshape(CT, P).T), wp=wp8,
                  w1=w1b, b2=f32(b2.reshape(HID // P, P).T),
                  wf2=wf2b, bf2=f32(bf2))
    in_maps = []
    for c in range(N_CORES):
        b, half = divmod(c, 2)
        own = x[b, N_OWN * half:N_OWN * (half + 1), :]
        oth = x[b, N_OWN * (1 - half):N_OWN * (2 - half), :]
        xp = np.concatenate([own, oth], axis=0)
        xrb = own + bp[None, :]
        in_maps.append({"x": xp.astype(NP_BF), "xrb": xrb.astype(NP_BF),
                        **shared})
    return in_maps


def run(inputs, trace=False, trace_kwargs=None):
    from concourse.bass_utils import run_bass_kernel_spmd
    nc = _get_nc()
    in_maps = _prep(inputs)
    res = run_bass_kernel_spmd(nc, in_maps, core_ids=list(range(N_CORES)),
                               trace=trace, **(trace_kwargs or {}))
    B = 4
    out = np.empty((B, N_ALL, C), dtype=np.float32)
    for c in range(N_CORES):
        b, half = divmod(c, 2)
        out[b, N_OWN * half:N_OWN * (half + 1), :] = res.results[c]["out"]
    return out, res


def kernel(**inputs):
    out, _ = run(inputs, trace=False)
    return out
